# revision 25
# baseline (speedup 1.0000x reference)
"""Trainium2 Bass kernel for nn_AstraloraLayer: y = (x @ W^T) * scale + x.

x: [16384, 1024] f32, w: [1048576] f32 (W = w.reshape(1024, 1024)),
scale: [1] f32.  Data-parallel over 8 NeuronCores: each core takes 2048
tokens; w and scale are replicated; no collectives needed.

Device layout: everything is computed transposed (y^T = W' @ x^T) so the
contraction dim d lands on SBUF partitions for both matmul operands with
zero on-device transposes.  The host passes x^T shards and W'^T where
W' = scale*W + I — folding the scalar scale AND the residual into the
weights makes the whole layer one matmul; the epilogue is a plain PSUM
drain (DVE copy + store).  Matmul operands are host-cast to bf16 (rel
err ~2e-3 vs the f32 reference; fp32 accumulation in PSUM), which also
halves input DMA traffic.

Block 0 runs k-outer across 8 PSUM banks so PE consumption matches DMA
arrival order (the first matmul waits on ~0.75 MB, not the 6 MB working
set); steady-state blocks run o-outer/k-inner so each output chunk's
PSUM drain pipelines behind the PE instead of bunching at block end.
A dozen throwaway matmuls on zeroed tiles pre-warm the PE's HAM clock
gate during the DMA lead-in.  w loads + y stores issue on the sync
HWDGE queue, x loads on the scalar HWDGE queue (DMA issue is ~0.6us
per 128-descriptor instruction — two queues double the feed rate).
"""

import numpy as np

_N_TOKENS = 16384
_D = 1024
_N_CORES = 8
_TOK_PER_CORE = _N_TOKENS // _N_CORES  # 2048
_TOK_BLOCK = 512
_P = 128

# Compute dtype for the matmul operands: "bf16" halves input DMA traffic
# (host casts the shards) and double-pumps the PE moving operand;
# "f32r" is full fp32 storage with single-pass reduced-precision matmul.
_COMPUTE = "bf16"

_cache = {}


def _apply_tile_drain_patch():
    """This walrus build rejects any instruction carrying more than one
    sync wait ("Too many sync wait commands", CoreV3 setupSyncWait), but
    Tile's wait-assignment pass freely emits multi-wait instructions.
    Two patches:

    1. Wrap TileClockWait so that after assign_waits() every instruction
       with >1 wait keeps only its last wait, with the others moved onto
       freshly inserted same-engine NoOps placed just before it.
    2. Re-emit the TileContext exit drain the same way (it waits on every
       live semaphore at once and is created after assign_waits ran).
    """
    if _cache.get("patched"):
        return
    import bass_rust
    import concourse.mybir as mybir
    from concourse import tile
    from concourse.vector_clock import ScopedClock

    _Orig = tile.TileClockWait
    _counter = [0]

    def _split_multi_waits(ordered):
        for insts in ordered.values():
            out = []
            for inst in insts:
                si = inst.sync_info
                if si is not None and len(si.on_wait) > 1:
                    waits = list(si.on_wait)
                    for w in waits[:-1]:
                        _counter[0] += 1
                        nop = mybir.InstNoOp(
                            name=f"I-wsplit-{_counter[0]}", ins=[], outs=[]
                        )
                        nop.engine = inst.engine
                        nop.bass_nofuse = True
                        nop.sync_info = bass_rust.SyncInfo(
                            on_wait=[w], on_update=[]
                        )
                        out.append(nop)
                    si.on_wait = waits[-1:]
                out.append(inst)
            insts[:] = out

    class _SplitWaitClock:
        def __init__(self, tc, ordered, **kw):
            object.__setattr__(self, "_inner", _Orig(tc, ordered, **kw))
            object.__setattr__(self, "_ordered", ordered)

        def assign_waits(self, bb):
            r = self._inner.assign_waits(bb)
            _split_multi_waits(self._ordered)
            return r

        def __getattr__(self, n):
            return getattr(object.__getattribute__(self, "_inner"), n)

    tile.TileClockWait = _SplitWaitClock

    def _drain_and_barrier(self, tick_clock, wait_clock):
        drain_inst = self.nc.sync.drain()
        wait_clock.add_sem_waits(
            drain_inst.ins, ScopedClock({None: tick_clock.global_clock})
        )
        si = drain_inst.ins.sync_info
        if si is not None and len(si.on_wait) > 1:
            waits = list(si.on_wait)
            si.on_wait = waits[:1]
            for w in waits[1:]:
                nop = self.nc.sync.nop(nofuse=True, hint="drain_wait_spill")
                nop.ins.sync_info = bass_rust.SyncInfo(on_wait=[w], on_update=[])

        self.nc.all_engine_barrier()
        assert self.sems is not None
        popped = self.nc._tile_sem_poison_stack.pop()
        assert popped is self._sem_poison
        self.nc.clear_and_free_semaphores(list(self.sems.allocated().values()))
        self.nc.all_engine_barrier()

    tile.TileContext._drain_and_barrier = _drain_and_barrier
    _cache["patched"] = True


def _build_nc(compute=None):
    import concourse.bass as bass
    import concourse.mybir as mybir
    from concourse import tile

    compute = compute or _COMPUTE
    f32 = mybir.dt.float32
    cd = mybir.dt.bfloat16 if compute == "bf16" else mybir.dt.float32r
    KC = _D // _P  # 8 contraction chunks
    OC = _D // _P  # 8 output-row chunks
    NB = _TOK_PER_CORE // _TOK_BLOCK  # token blocks

    nc = bass.Bass()
    xT = nc.declare_dram_parameter("xT", [_D, _TOK_PER_CORE], cd, isOutput=False)
    wT = nc.declare_dram_parameter("wT", [_D, _D], cd, isOutput=False)
    yT = nc.declare_dram_parameter("yT", [_D, _TOK_PER_CORE], f32, isOutput=True)

    with tile.TileContext(nc) as tc:
        with (
            tc.tile_pool(name="wp", bufs=1) as wp,
            tc.tile_pool(name="xp", bufs=2) as xp,
            tc.tile_pool(name="yp", bufs=4) as yp,
            tc.tile_pool(name="ps", bufs=1, space="PSUM") as ps,
        ):
            # PE pre-warm: the HAM clock gate holds the PE at 1.2 GHz until
            # it has been busy ~3.4us.  The PE would otherwise sit idle
            # through the NEFF preamble + first DMAs and then run its first
            # ~40 real matmuls at half clock.  Feed it throwaway matmuls on
            # zeroed tiles so it is at 2.4 GHz when the real stream starts.
            warm_dt = mybir.dt.bfloat16  # f32r memset is invalid ISA
            warm_w = wp.tile([_P, _P], warm_dt, tag="warm_w")
            warm_x = wp.tile([_P, _TOK_BLOCK], warm_dt, tag="warm_x")
            nc.vector.memset(warm_w[:], 0.0)
            nc.vector.memset(warm_x[:], 0.0)
            warm_ps = ps.tile([_P, _TOK_BLOCK], f32, tag="ps7", name="warm_ps")
            for i in range(11):
                nc.tensor.matmul(
                    warm_ps[:], lhsT=warm_w[:], rhs=warm_x[:],
                    start=True, stop=True,
                )

            # Weights: 4 DMAs of two k-chunks each on the sync HWDGE queue.
            # DMA issue is ~0.6us per 128-descriptor instruction, so bigger
            # rows (not more instructions) is how the feed keeps up.
            wpairs = []
            for j in range(KC // 2):
                wt = wp.tile([_P, 2 * _D], cd, tag=f"w{j}")
                nc.sync.dma_start(
                    out=wt[:].rearrange("p (two d) -> p two d", two=2),
                    in_=wT[2 * j * _P : (2 * j + 2) * _P, :].rearrange(
                        "(two p) d -> p two d", two=2
                    ),
                )
                wpairs.append(wt)

            def w_slice(k, o):
                return wpairs[k // 2][
                    :, (k % 2) * _D + o * _P : (k % 2) * _D + (o + 1) * _P
                ]

            # x: per k, one DMA covering TWO token blocks (2KB rows) on the
            # scalar HWDGE queue so load issue runs parallel to the w queue.
            xtiles = {}  # (bpair, k) -> tile
            for b in range(NB):
                t0 = b * _TOK_BLOCK
                bp, half = divmod(b, 2)
                if half == 0:
                    for k in range(KC):
                        t = xp.tile(
                            [_P, 2 * _TOK_BLOCK], cd, tag=f"x{k}", name=f"x{k}_{bp}"
                        )
                        nc.scalar.dma_start(
                            out=t[:],
                            in_=xT[
                                k * _P : (k + 1) * _P, t0 : t0 + 2 * _TOK_BLOCK
                            ],
                        )
                        xtiles[(bp, k)] = t

                def x_slice(k):
                    return xtiles[(bp, k)][
                        :, half * _TOK_BLOCK : (half + 1) * _TOK_BLOCK
                    ]

                def epilogue(o, pt):
                    # scale and residual are folded into the host weights
                    # (W' = scale*W + I), so the epilogue is a plain PSUM
                    # drain: DVE copy to SBUF, gpsimd SWDGE store.
                    yt = yp.tile([_P, _TOK_BLOCK], f32, tag="y", name=f"y{o}_{b}")
                    nc.vector.tensor_copy(yt[:], pt[:])
                    nc.sync.dma_start(
                        out=yT[o * _P : (o + 1) * _P, t0 : t0 + _TOK_BLOCK],
                        in_=yt[:],
                    )

                if b == 0:
                    # k-outer for the first block: consumption order matches
                    # DMA arrival order (w_k + x_k per step), so the PE
                    # starts after ~0.75 MB instead of the full working set.
                    pts = [
                        ps.tile([_P, _TOK_BLOCK], f32, tag=f"ps{o}", name=f"ps{o}_0")
                        for o in range(OC)
                    ]
                    for k in range(KC):
                        for o in range(OC):
                            nc.tensor.matmul(
                                pts[o][:],
                                lhsT=w_slice(k, o),
                                rhs=x_slice(k),
                                start=(k == 0),
                                stop=(k == KC - 1),
                            )
                            if k == KC - 1:
                                epilogue(o, pts[o])
                else:
                    # o-outer / k-inner for steady state: each 128-row
                    # output chunk finishes every 8 matmuls, so its PSUM
                    # drain pipelines behind the PE instead of bunching up
                    # after the block's last matmul.
                    for o in range(OC):
                        pt = ps.tile(
                            [_P, _TOK_BLOCK], f32, tag=f"ps{o}", name=f"ps{o}_{b}"
                        )
                        for k in range(KC):
                            nc.tensor.matmul(
                                pt[:],
                                lhsT=w_slice(k, o),
                                rhs=x_slice(k),
                                start=(k == 0),
                                stop=(k == KC - 1),
                            )
                        epilogue(o, pt)
    return nc


def _np_compute_dtype():
    if _COMPUTE == "bf16":
        import ml_dtypes

        return ml_dtypes.bfloat16
    return np.float32


def kernel(x, w, scale):
    _apply_tile_drain_patch()
    from concourse.bass_utils import run_bass_kernel_spmd

    x = np.asarray(x, dtype=np.float32)
    w = np.asarray(w, dtype=np.float32)
    scale = np.asarray(scale, dtype=np.float32).reshape(1)
    cdt = _np_compute_dtype()
    # Fold the scalar scale and the residual identity into the weights:
    # y^T = (scale*W + I) @ x^T  ==  (scale * (x @ W^T) + x)^T  exactly.
    wTp = (w.reshape(_D, _D).T * scale[0] + np.eye(_D, dtype=np.float32)).astype(cdt)

    in_maps = []
    for i in range(_N_CORES):
        xs = x[i * _TOK_PER_CORE : (i + 1) * _TOK_PER_CORE]
        in_maps.append(
            {
                "xT": np.ascontiguousarray(xs.T).astype(cdt),
                "wT": wTp,
            }
        )

    if "nc" not in _cache:
        _cache["nc"] = _build_nc()
    res = run_bass_kernel_spmd(_cache["nc"], in_maps, core_ids=list(range(_N_CORES)))

    out = np.empty((_N_TOKENS, _D), dtype=np.float32)
    for i in range(_N_CORES):
        out[i * _TOK_PER_CORE : (i + 1) * _TOK_PER_CORE] = res.results[i]["yT"].T
    return out
